# revision 90
# baseline (speedup 1.0000x reference)
"""GAT (2-layer, 3-head) forward on 8 Trainium2 NeuronCores.

Math: with LeakyReLU slope ALPHA=1.0 the edge score e_ij = s1_i + s2_j is
linear, and s1_i cancels inside the row softmax.  The masked softmax over
j therefore reduces to column weights w_j = exp(s2_j - C) restricted to
adj, giving

    h'_i = (sum_j adj_ij * w_j * h_j) / (sum_j adj_ij * w_j)

i.e. one adjacency matmul against G = [w*h | w].  Both GAT layers take
this form (the same adjacency masks both), so the whole network is two
A-matmuls plus small projections.

Sharding: rows of h' (nodes) across 8 cores; each core holds lhsT-layout
adjacency columns A^T[:, slab] and computes its 512-row slab.

Precision (tolerance 2e-2, achieved ~4e-3): the layer-1 adjacency matmul
runs in fp8 e4m3 with DoubleRow perf mode (each matmul contracts a
256-node j-pair at 2x rate); G is scaled by 8, the denominator weights w
by 128 as an fp8 hi/lo pair (hi + lo/16 ~ 8 mantissa bits), with the
scale ratio folded into the reciprocal.  x@W, layer 2 and the epilogues
are bf16.  The edge-score s2, which sits in an exponent, uses a bf16
hi/lo pair of the folded u vector; its softmax max is computed exactly
on the host (negC input), and layer 2 skips max subtraction entirely
(s2' <= ~8, the common exp scale cancels in num/den).

Schedule: batched input DMAs; head-0 x@W + w staging feed gather A
[w-pair cols | G0] so the denominator (riding ct-0's tile loads) and
head-0 column-tiles unblock first, while gather B [G1 | G2] overlaps the
head-0 adjacency matmuls.  h2 accumulates inline per column-tile; g2 is
gathered in two half-slab chunks so the L2 matmul starts on the first
half; the final elu+log_softmax runs batched over all four i-tiles.
"""
import sys

sys.path.insert(0, "/opt/trn_rl_repo")

import numpy as np
import ml_dtypes

import concourse.bass as bass
import concourse.bacc as bacc
import concourse.mybir as mybir
import concourse.bass_isa as bass_isa
import concourse.tile as tile
from concourse.bass_utils import run_bass_kernel_spmd

BF16 = ml_dtypes.bfloat16
F8E4 = ml_dtypes.float8_e4m3

N = 4096
F = 768
HID = 768
NH = 3
NCLS = 256
NCORES = 8
SLAB = N // NCORES          # 512 rows per core
NIT = SLAB // 128           # 4 i-tiles per core
NJT = N // 128              # 32 j-tiles
NFT = F // 128              # 6 f-tiles
NCT = NH * NFT              # 18 feature col-tiles of G
G2C = NCLS + 1              # 257 = classes + s2' column (folded u2)
PAD2 = 264                  # G2 padded to 32B rows
WCOLS = 32                  # w-column slab width (6 used + pad, 32B rows)
GA = WCOLS + HID            # gather-A width: [w cols | head0 G]
GB = 2 * HID                # gather-B width: head1 + head2 G
GH_TOT = NH * HID           # 2304 xcat feature rows of Wo
SG = 8.0                    # fp8 scale on G ( |G*8| << 240 )
SW = 128.0                  # fp8 scale on w (w <= 1)
NJJ = NJT // 2              # 16 j-pair blocks for DoubleRow

AF = mybir.ActivationFunctionType
ALU = mybir.AluOpType


def _enable_ldw_opt():
    # walrus defaults to --enable-ldw-opt=false; with it off every LDWEIGHTS
    # serializes against the previous matmul (~427ns vs ~213ns per 512-col
    # matmul).  Patch the arg builder so the stationary loads pipeline.
    import concourse.bass_utils as _bu
    if getattr(_bu, "_ldw_opt_patched", False):
        return
    _orig = _bu.get_walrus_args

    def _patched(*a, **k):
        args = _orig(*a, **k)
        return [x.replace("--enable-ldw-opt=false", "--enable-ldw-opt=true")
                for x in args]

    _bu.get_walrus_args = _patched
    _bu._ldw_opt_patched = True


def build():
    dt = mybir.dt
    _enable_ldw_opt()
    nc = bacc.Bacc(num_devices=NCORES)

    adjT8_d = nc.dram_tensor("adjT8", [N, SLAB], dt.float8e4, kind="ExternalInput")
    adjT_d = nc.dram_tensor("adjT", [N, SLAB], dt.bfloat16, kind="ExternalInput")
    xTh_d = nc.dram_tensor("xT_hi", [F, SLAB], dt.bfloat16, kind="ExternalInput")
    U6_d = nc.dram_tensor("U6", [F, 8], dt.bfloat16, kind="ExternalInput")
    # negC[0, h] = -max_i s2_i(head h), computed exactly on the host
    negC_d = nc.dram_tensor("negC", [1, NH], dt.float32, kind="ExternalInput")
    W_d = nc.dram_tensor("W", [NH, F, HID], dt.bfloat16, kind="ExternalInput")
    Wo_d = nc.dram_tensor("Wo", [GH_TOT, G2C], dt.bfloat16, kind="ExternalInput")
    out_d = nc.dram_tensor("out", [SLAB, NCLS], dt.float32, kind="ExternalOutput")

    # DRAM scratch + collective buffers (fp8: halves gather + reload bytes)
    gsA = nc.dram_tensor("gsA", [SLAB, GA], dt.float8e4)
    gfA = nc.dram_tensor("gfA", [N, GA], dt.float8e4, addr_space="Shared")
    gsB = nc.dram_tensor("gsB", [SLAB, GB], dt.float8e4)
    gfB = nc.dram_tensor("gfB", [N, GB], dt.float8e4, addr_space="Shared")
    # g2 gathered in two half-slab chunks so the L2 matmul can start on the
    # first half while the second is in flight
    g2_slab = [nc.dram_tensor(f"g2_slab{k}", [SLAB // 2, PAD2], dt.bfloat16)
               for k in range(2)]
    g2_full = [nc.dram_tensor(f"g2_full{k}", [N // 2, PAD2], dt.bfloat16,
                              addr_space="Shared") for k in range(2)]

    rg = [list(range(NCORES))]

    with tile.TileContext(nc) as tc:
      with tc.tile_pool(name="adjt", bufs=NJT) as p_adjt:
        # ---------------- phase 1: s2, w, h=x@W, G build + gathers ----------
        with (
            tc.tile_pool(name="xw", bufs=1) as p_xw,
            tc.tile_pool(name="small", bufs=1) as p_sm,
            tc.tile_pool(name="gtmp", bufs=1) as p_gt,
        ):
            # Batched input loads: one big DMA per tensor (chunked transfers
            # serialize at ~650ns per 128KB, so 70 small DMAs would cost
            # ~45us of serial load time).  x + head-0 W first: they gate
            # s2 and the first x@W matmuls.
            xh_all = p_xw.tile([128, NFT, SLAB], dt.bfloat16, tag="xh", name="xh")
            nc.sync.dma_start(xh_all[:],
                              xTh_d.rearrange("(ft p) i -> p ft i", p=128))

            def xhi(ft, c0, c1):
                return xh_all[:, ft, c0:c1]

            u6 = p_sm.tile([128, NFT, 8], dt.bfloat16, tag="u6", name="u6")
            nc.gpsimd.dma_start(u6[:], U6_d.rearrange("(ft p) c -> p ft c", p=128))
            negC = p_sm.tile([1, NH], dt.float32, tag="negC", name="negC")
            nc.gpsimd.dma_start(negC[:], negC_d[:])
            negCbc = p_sm.tile([128, NH], dt.float32, tag="negCbc", name="negCbc")
            nc.gpsimd.partition_broadcast(negCbc[:], negC[:], channels=128)

            W_t = W_d.rearrange("h (ft p) o -> p h ft o", p=128)
            w0_all = p_xw.tile([128, NFT, HID], dt.bfloat16, tag="w0", name="w0")
            nc.sync.dma_start(w0_all[:], W_t[:, 0])
            w12_all = p_xw.tile([128, 2, NFT, HID], dt.bfloat16, tag="w12",
                                name="w12")
            nc.scalar.dma_start(w12_all[:], W_t[:, 1:3])

            def wsl(h, ft, c0, c1):
                if h == 0:
                    return w0_all[:, ft, c0:c1]
                return w12_all[:, h - 1, ft, c0:c1]

            # fp8 adjacency, j-pair interleaved for DoubleRow (L1 rhs)
            adj8_all = []
            adjT8_t = adjT8_d.rearrange("(half jj i p) n -> half p jj i n",
                                        half=2, i=2, p=128)
            for half in range(2):
                t = p_adjt.tile([128, NJJ // 2, 2, SLAB], dt.float8e4,
                                tag="adj8", name="adj8", bufs=2)
                eng = nc.sync if half == 0 else nc.scalar
                eng.dma_start(t[:], adjT8_t[half])
                adj8_all.append(t)

            def adjd(jj):
                return adj8_all[jj // (NJJ // 2)][:, jj % (NJJ // 2), :, :]

            # bf16 adjacency per original j-tile (L2 lhsT) — needed only at
            # the tail, loaded after the phase-1 traffic
            adjt_all = []
            adjT_t = adjT_d.rearrange("(half jh p) i -> half p jh i",
                                      half=2, p=128)

            def adjs(j, c0=0, c1=SLAB):
                return adjt_all[j // (NJT // 2)][:, j % (NJT // 2), c0:c1]

            # head-0 x@W its 0-2 FIRST (PE warms up, psum tiles park until w
            # is ready), then the tiny s2 matmuls — their DVE chain and the
            # exp overlap the head-0 compute, so gather A fires earlier.
            ctx_psA = tc.tile_pool(name="psA", bufs=3, space="PSUM")
            ps_a = ctx_psA.__enter__()

            def xw_head(h, it):
                ps = ps_a.tile([128, HID], dt.float32, tag="psA", name="psA")
                for ft in range(NFT):
                    xh = xhi(ft, it * 128, (it + 1) * 128)
                    nc.tensor.matmul(ps[:, 0:512], xh, wsl(h, ft, 0, 512),
                                     start=(ft == 0), stop=(ft == NFT - 1))
                    nc.tensor.matmul(ps[:, 512:HID], xh, wsl(h, ft, 512, HID),
                                     start=(ft == 0), stop=(ft == NFT - 1))
                return ps

            h0_ps = [xw_head(0, it) for it in range(NIT - 1)]

            # s2 = x_hi @ (u_hi + u_lo): one PSUM bank, no inter-it reuse
            # stalls.  u kept as a bf16 pair; x_hi-only costs ~0.8% on w,
            # which averages out over ~2k neighbours.
            s2_sb = []
            for h in range(NH):
                s2_sb.append(p_sm.tile([128, NIT], dt.float32, tag="s2",
                                       name="s2", bufs=NH))
            with tc.tile_pool(name="psS", bufs=1, space="PSUM") as ps_s:
                p6 = ps_s.tile([128, NIT, 8], dt.float32, tag="p6", name="p6")
                for it in range(NIT):
                    for ft in range(NFT):
                        xh = xhi(ft, it * 128, (it + 1) * 128)
                        nc.tensor.matmul(p6[:, it, :], xh, u6[:, ft, :],
                                         start=(ft == 0), stop=(ft == NFT - 1))
                for it in range(NIT):
                    t6 = p_sm.tile([128, 8], dt.float32, tag="t6", name="t6",
                                   bufs=2)
                    nc.vector.tensor_copy(t6[:], p6[:, it, :])
                    tsum = p_sm.tile([128, NH], dt.float32, tag="tsum",
                                     name="tsum", bufs=2)
                    nc.vector.tensor_tensor(tsum[:], t6[:, 0:2 * NH:2],
                                            t6[:, 1:2 * NH:2], ALU.add)
                    for h in range(NH):
                        nc.vector.tensor_copy(s2_sb[h][:, it:it + 1],
                                              tsum[:, h:h + 1])

            # w = exp(s2 - C) with the host-computed C — no collective needed.
            # Stage w*SW as an fp8 hi/lo pair (hi + lo/16 ≈ 8 mantissa bits)
            # for the DoubleRow denominator matmul, and keep w*SG in fp32 for
            # scaling G.
            w_sb, w8_sb = [], []
            for h in range(NH):
                w = p_sm.tile([128, NIT], dt.float32, tag="wexp", name="wexp",
                              bufs=NH)
                nc.scalar.activation(w[:], s2_sb[h][:], AF.Exp,
                                     bias=negCbc[:, h:h + 1])
                w_sb.append(w)
                w8 = p_sm.tile([128, NIT], dt.float32, tag="wsg", name="wsg",
                               bufs=NH)
                nc.vector.tensor_scalar_mul(w8[:], w[:], SG)
                w8_sb.append(w8)
            whi3 = p_sm.tile([128, NH, NIT], dt.float8e4, tag="whi3",
                             name="whi3")
            wlo3 = p_sm.tile([128, NH, NIT], dt.float8e4, tag="wlo3",
                             name="wlo3")
            for h in range(NH):
                wsw = p_sm.tile([128, NIT], dt.float32, tag="wsw", name="wsw",
                                bufs=2)
                nc.vector.tensor_scalar_mul(wsw[:], w_sb[h][:], SW)
                nc.vector.tensor_copy(whi3[:, h, :], wsw[:])
                wr = p_sm.tile([128, NIT], dt.float32, tag="wr", name="wr",
                               bufs=2)
                nc.vector.tensor_tensor(wr[:], wsw[:], whi3[:, h, :],
                                        ALU.subtract)
                nc.vector.tensor_scalar_mul(wlo3[:, h, :], wr[:], 16.0)
            for it in range(NIT):
                rows = slice(it * 128, (it + 1) * 128)
                wt = p_sm.tile([128, WCOLS], dt.float8e4, tag="wt", name="wt",
                               bufs=2)
                nc.vector.memset(wt[:], 0.0)
                nc.vector.tensor_copy(wt[:, 0:NH], whi3[:, :, it])
                nc.vector.tensor_copy(wt[:, NH:2 * NH], wlo3[:, :, it])
                nc.gpsimd.dma_start(gsA[rows, 0:WCOLS], wt[:])

            # head-0 G build -> gather A fires as early as possible
            for it in range(NIT - 1):
                g = p_gt.tile([128, HID], dt.float8e4, tag="g0",
                              name="g0", bufs=4)
                nc.vector.tensor_scalar_mul(g[:], h0_ps[it][:],
                                            w8_sb[0][:, it:it + 1])
                rows = slice(it * 128, (it + 1) * 128)
                nc.sync.dma_start(gsA[rows, WCOLS:GA], g[:])
            ps = xw_head(0, NIT - 1)
            g = p_gt.tile([128, HID], dt.float8e4, tag="g0", name="g0", bufs=4)
            nc.vector.tensor_scalar_mul(g[:], ps[:], w8_sb[0][:, NIT - 1:NIT])
            nc.sync.dma_start(gsA[(NIT - 1) * 128:SLAB, WCOLS:GA], g[:])
            nc.gpsimd.collective_compute(
                "AllGather", ALU.bypass, replica_groups=rg,
                ins=[gsA[:]], outs=[gfA[:]])
            h0_ps = None

            # heads 1-2: x@W, scale, stage, gather B
            for h in (1, 2):
                for it in range(NIT):
                    ps = xw_head(h, it)
                    g = p_gt.tile([128, HID], dt.float8e4, tag="g0",
                                  name="g0", bufs=4)
                    nc.vector.tensor_scalar_mul(g[:], ps[:],
                                                w8_sb[h][:, it:it + 1])
                    rows = slice(it * 128, (it + 1) * 128)
                    eng = nc.sync if h == 1 else nc.scalar
                    eng.dma_start(gsB[rows, (h - 1) * HID:h * HID], g[:])
            nc.gpsimd.collective_compute(
                "AllGather", ALU.bypass, replica_groups=rg,
                ins=[gsB[:]], outs=[gfB[:]])
            ctx_psA.__exit__(None, None, None)
            # bf16 adjacency for the L2 lhsT — queue after the G staging
            for half in range(2):
                t = p_adjt.tile([128, NJT // 2, SLAB], dt.bfloat16, tag="adjt",
                                name="adjt", bufs=2)
                eng = nc.sync if half == 0 else nc.scalar
                eng.dma_start(t[:], adjT_t[half])
                adjt_all.append(t)

        # ---------------- L1 adjacency matmul + epilogue + layer 2 ----------
        with tc.tile_pool(name="xct", bufs=1) as p_xct:
            with (
                tc.tile_pool(name="gst", bufs=8) as p_gst,
                tc.tile_pool(name="etmp", bufs=1) as p_et,
                tc.tile_pool(name="wo", bufs=1) as p_wo,
                tc.tile_pool(name="l2a", bufs=1) as p_l2a,
                tc.tile_pool(name="ps1", bufs=4, space="PSUM") as ps_1,
                tc.tile_pool(name="psh2", bufs=4, space="PSUM") as ps_h2,
            ):
                # Wo loads early; they only feed the inline h2 matmuls
                wo_sb = []
                Wo_t = Wo_d.rearrange("(ot p) c -> ot p c", p=128)
                for ot in range(NCT):
                    t = p_wo.tile([128, G2C], dt.bfloat16, tag="wo", name="wo",
                                  bufs=NCT)
                    eng = nc.sync if ot % 2 == 0 else nc.scalar
                    eng.dma_start(t[:], Wo_t[ot])
                    wo_sb.append(t)

                # feature col-tiles, head-major; epilogue + h2 inline per ct.
                # DoubleRow fp8: each matmul contracts a j-PAIR (256 nodes).
                # ct 0 also carries the w columns (first WCOLS of gfA), so the
                # denominator matmuls ride its tile loads — no separate
                # strided gather of w.
                gvA = gfA.rearrange("(jb jj i p) c -> jb p jj i c",
                                    jj=2, i=2, p=128)
                gvB = gfB.rearrange("(jb jj i p) c -> jb p jj i c",
                                    jj=2, i=2, p=128)
                ps2l = [ps_h2.tile([128, G2C], dt.float32, tag="psh2",
                                   name="psh2") for _ in range(NIT)]
                rbc = [None] * NH
                psd = ps_1.tile([NH, 2, SLAB], dt.float32, tag="psd",
                                name="psd", bufs=1)
                DR = mybir.MatmulPerfMode.DoubleRow
                for cp in range(NCT // 2):
                    h = (2 * cp) // NFT
                    lp = cp % (NFT // 2)
                    pss = [ps_1.tile([128, SLAB], dt.float32, tag="ps1",
                                     name="ps1", bufs=2) for _ in range(2)]
                    for jb in range(NJJ // 2):
                        # one load covers BOTH cts of the pair (256B chunks)
                        if cp == 0:
                            gt = p_gst.tile([128, 2, 2, WCOLS + 256],
                                            dt.float8e4, tag="gst0",
                                            name="gst0", bufs=8)
                            eng = nc.sync if jb % 2 == 0 else nc.scalar
                            eng.dma_start(gt[:], gvA[jb, :, :, :, 0:WCOLS + 256])
                            goff = WCOLS
                        elif h == 0:
                            gt = p_gst.tile([128, 2, 2, 256], dt.float8e4,
                                            tag="gst", name="gst")
                            eng = nc.sync if jb % 2 == 0 else nc.scalar
                            eng.dma_start(gt[:], gvA[jb, :, :, :,
                                                     WCOLS + lp * 256:
                                                     WCOLS + (lp + 1) * 256])
                            goff = 0
                        else:
                            gt = p_gst.tile([128, 2, 2, 256], dt.float8e4,
                                            tag="gst", name="gst")
                            eng = nc.sync if jb % 2 == 0 else nc.scalar
                            c0 = (h - 1) * HID + lp * 256
                            eng.dma_start(gt[:], gvB[jb, :, :, :, c0:c0 + 256])
                            goff = 0
                        for q in range(2):
                            jj = jb * 2 + q
                            if cp == 0:
                                nc.tensor.matmul(psd[:, 0, :],
                                                 gt[:, q, :, 0:NH],
                                                 adjd(jj), start=(jj == 0),
                                                 stop=(jj == NJJ - 1),
                                                 perf_mode=DR)
                                nc.tensor.matmul(psd[:, 1, :],
                                                 gt[:, q, :, NH:2 * NH],
                                                 adjd(jj), start=(jj == 0),
                                                 stop=(jj == NJJ - 1),
                                                 perf_mode=DR)
                            for s in range(2):
                                o = goff + s * 128
                                nc.tensor.matmul(pss[s][:],
                                                 gt[:, q, :, o:o + 128],
                                                 adjd(jj), start=(jj == 0),
                                                 stop=(jj == NJJ - 1),
                                                 perf_mode=DR)
                    if cp == 0:
                        # den = (psd_hi + psd_lo/16); recip carries SW/SG
                        dlo = p_et.tile([NH, SLAB], dt.float32, tag="dlo",
                                        name="dlo")
                        nc.vector.tensor_copy(dlo[:], psd[:, 1, :])
                        den3 = p_et.tile([NH, SLAB], dt.float32, tag="den3",
                                         name="den3")
                        nc.vector.scalar_tensor_tensor(den3[:], dlo[:],
                                                       1.0 / 16.0, psd[:, 0, :],
                                                       ALU.mult, ALU.add)
                        recip3 = p_et.tile([NH, SLAB], dt.float32, tag="recip3",
                                           name="recip3")
                        nc.vector.reciprocal(recip3[:], den3[:])
                        nc.vector.tensor_scalar_mul(recip3[:], recip3[:],
                                                    SW / SG)
                        for hh in range(NH):
                            rrow = p_et.tile([1, SLAB], dt.float32, tag="rrow",
                                             name="rrow", bufs=2)
                            nc.sync.dma_start(rrow[:], recip3[hh:hh + 1, :])
                            rb = p_et.tile([128, SLAB], dt.float32, tag="rbc",
                                           name="rbc", bufs=NH)
                            nc.gpsimd.partition_broadcast(rb[:], rrow[:],
                                                          channels=128)
                            rbc[hh] = rb
                    for s in range(2):
                        ct = 2 * cp + s
                        # xcatT tile = elu(numT / den), bf16
                        z = p_et.tile([128, SLAB], dt.float32, tag="z",
                                      name="z", bufs=2)
                        nc.vector.tensor_tensor(z[:], pss[s][:], rbc[h][:],
                                                ALU.mult)
                        e = p_et.tile([128, SLAB], dt.float32, tag="e",
                                      name="e", bufs=2)
                        nc.scalar.activation(e[:], z[:], AF.Exp)
                        nc.vector.tensor_scalar(e[:], e[:], 1.0, -1.0, ALU.min,
                                                ALU.add)
                        xc = p_xct.tile([128, SLAB], dt.bfloat16, tag="xcp",
                                        name="xcp", bufs=NCT)
                        nc.vector.scalar_tensor_tensor(xc[:], z[:], 0.0, e[:],
                                                       ALU.max, ALU.add)
                        # layer 2 accumulation: h2 += xcat_ct @ Wo_ct
                        for it in range(NIT):
                            nc.tensor.matmul(ps2l[it][:],
                                             xc[:, it * 128:(it + 1) * 128],
                                             wo_sb[ct][:],
                                             start=(ct == 0),
                                             stop=(ct == NCT - 1))

                # layer-2 weights w2 = exp(s2') with NO max subtraction:
                # s2' stays well under fp32/bf16 exp range and the common
                # scale cancels exactly in num/den.
                for it in range(NIT):
                    rows = slice((it % 2) * 128, (it % 2 + 1) * 128)
                    w2 = p_l2a.tile([128, 1], dt.float32, tag="w2", name="w2",
                                    bufs=2)
                    nc.scalar.activation(w2[:], ps2l[it][:, NCLS:G2C], AF.Exp)
                    g2b = p_l2a.tile([128, PAD2], dt.bfloat16, tag="g2b",
                                     name="g2b", bufs=2)
                    nc.vector.tensor_scalar_mul(g2b[:, 0:NCLS],
                                                ps2l[it][:, 0:NCLS], w2[:])
                    nc.vector.tensor_copy(g2b[:, NCLS:G2C], w2[:])
                    nc.vector.memset(g2b[:, G2C:PAD2], 0.0)
                    nc.sync.dma_start(g2_slab[it // 2][rows, :], g2b[:])
                    if it == 1:
                        nc.gpsimd.collective_compute(
                            "AllGather", ALU.bypass, replica_groups=rg,
                            ins=[g2_slab[0][:]], outs=[g2_full[0][:]])
                nc.gpsimd.collective_compute(
                    "AllGather", ALU.bypass, replica_groups=rg,
                    ins=[g2_slab[1][:]], outs=[g2_full[1][:]])

            # L2 adjacency matmul + final epilogue
            with (
                tc.tile_pool(name="g2t", bufs=NJT) as p_g2t,
                tc.tile_pool(name="fin", bufs=1) as p_f,
                tc.tile_pool(name="ps2", bufs=4, space="PSUM") as ps_2,
            ):
                g2tiles = []
                for k in range(2):
                    g2v = g2_full[k].rearrange("(tb t p) c -> tb p t c",
                                               tb=2, p=128)
                    for tb in range(2):
                        gt2 = p_g2t.tile([128, 8, PAD2], dt.bfloat16,
                                         tag="g2t", name="g2t", bufs=4)
                        eng = nc.sync if tb % 2 == 0 else nc.scalar
                        eng.dma_start(gt2[:], g2v[tb])
                        g2tiles.append(gt2)
                # one psum tile, 512-col (bank-aligned) stride per it
                ps2 = ps_2.tile([128, NIT, 512], dt.float32, tag="ps2",
                                name="ps2", bufs=1)
                for k in range(2):
                    for it in range(NIT):
                        for t in range(NJT // 2):
                            jt = (t // 2) * 4 + k * 2 + (t % 2)
                            lhs = adjs(jt, it * 128, (it + 1) * 128)
                            g2idx = k * 2 + t // 8
                            nc.tensor.matmul(
                                ps2[:, it, 0:G2C], lhs,
                                g2tiles[g2idx][:, t % 8, 0:G2C],
                                start=(k == 0 and t == 0),
                                stop=(k == 1 and t == NJT // 2 - 1))
                # batched final epilogue: elu + log_softmax on all 4 i-tiles
                # at once (o <= ~10, so exp needs no max subtraction)
                z4 = p_f.tile([128, NIT, NCLS], dt.float32, tag="z4", name="z4")
                for it in range(NIT):
                    r2 = p_f.tile([128, 1], dt.float32, tag="r2", name="r2",
                                  bufs=2)
                    nc.vector.reciprocal(r2[:], ps2[:, it, NCLS:G2C])
                    nc.vector.tensor_scalar_mul(z4[:, it, :],
                                                ps2[:, it, 0:NCLS], r2[:])
                e4 = p_f.tile([128, NIT, NCLS], dt.float32, tag="e4", name="e4")
                nc.scalar.activation(e4[:], z4[:], AF.Exp)
                nc.vector.tensor_scalar(e4[:], e4[:], 1.0, -1.0, ALU.min,
                                        ALU.add)
                o4 = p_f.tile([128, NIT, NCLS], dt.float32, tag="o4", name="o4")
                nc.vector.scalar_tensor_tensor(o4[:], z4[:], 0.0, e4[:],
                                               ALU.max, ALU.add)
                t4 = p_f.tile([128, NIT, NCLS], dt.float32, tag="t4", name="t4")
                nc.scalar.activation(t4[:], o4[:], AF.Exp)
                ssum4 = p_f.tile([128, NIT, 1], dt.float32, tag="ssum4",
                                 name="ssum4")
                nc.vector.tensor_reduce(ssum4[:], t4[:],
                                        axis=mybir.AxisListType.X, op=ALU.add)
                lg4 = p_f.tile([128, NIT, 1], dt.float32, tag="lg4", name="lg4")
                nc.scalar.activation(lg4[:], ssum4[:], AF.Ln)
                fin4 = p_f.tile([128, NIT, NCLS], dt.float32, tag="fin4",
                                name="fin4")
                for it in range(NIT):
                    nc.vector.tensor_scalar(fin4[:, it, :], o4[:, it, :],
                                            lg4[:, it, :], None, ALU.subtract)
                nc.sync.dma_start(
                    out_d.rearrange("(it p) c -> p it c", p=128), fin4[:])

    nc.finalize()
    return nc


_CACHE = {}


def _pair(a):
    hi = a.astype(BF16)
    lo = (a - hi.astype(np.float32)).astype(BF16)
    return hi, lo


def prepare_inputs(x, adj, W_heads, a_heads, W_out, a_out):
    """Shard + lay out the full inputs for the 8 cores."""
    x2 = np.asarray(x, np.float32)[0]          # [N, F]
    adj2 = np.asarray(adj)[0]                  # [N, N] int32
    W3 = np.asarray(W_heads, np.float32).reshape(NH, F, HID)
    a3 = np.asarray(a_heads, np.float32)       # [NH, 2*HID, 1]
    Wo = np.asarray(W_out, np.float32).reshape(GH_TOT, NCLS)
    ao = np.asarray(a_out, np.float32)         # [2*NCLS, 1]

    # fold the edge-score projections into the weights:
    #   s2 = x @ (W @ a2),   s2' = xcat @ (Wo @ ao2)
    u = np.einsum("hfo,ho->hf", W3.astype(np.float64),
                  a3[:, HID:, 0].astype(np.float64)).astype(np.float32)  # [NH,F]
    u_hi, u_lo = _pair(u)
    U6 = np.zeros((F, 8), BF16)
    for h in range(NH):
        U6[:, 2 * h] = u_hi[h]
        U6[:, 2 * h + 1] = u_lo[h]
    u2 = (Wo.astype(np.float64) @ ao[NCLS:, 0].astype(np.float64)).astype(np.float32)
    Wo_ext = np.concatenate([Wo, u2[:, None]], axis=1)       # [GH, 257]
    Wo_b = Wo_ext.astype(BF16)
    W_b = W3.astype(BF16)
    xT = np.ascontiguousarray(x2.T)            # [F, N]
    adjb = adj2.astype(BF16)                   # exact 0/1

    # exact per-head max of s2 = x @ u, folded on the host so the device
    # needs no max-reduction collective.  Mirror the device arithmetic
    # (bf16 x_hi against the u hi/lo pair, accumulated in fp32).
    xh_f = x2.astype(BF16).astype(np.float32)
    s2 = (xh_f @ u_hi.T.astype(np.float32)
          + xh_f @ u_lo.T.astype(np.float32))                     # [N, NH]
    negC = -s2.max(axis=0, keepdims=True).astype(np.float32)      # [1, NH]

    in_maps = []
    for c in range(NCORES):
        sl = slice(c * SLAB, (c + 1) * SLAB)
        xh = np.ascontiguousarray(xT[:, sl]).astype(BF16)
        adjTc = np.ascontiguousarray(adjb[sl, :].T)
        in_maps.append({
            "adjT": adjTc,
            "adjT8": adjTc.astype(F8E4),
            "xT_hi": xh,
            "U6": U6, "negC": negC,
            "W": W_b, "Wo": Wo_b,
        })
    return in_maps


def kernel(x, adj, W_heads, a_heads, W_out, a_out):
    if "nc" not in _CACHE:
        # touch the devices once so any residual bad state from a previous
        # process surfaces (and clears) before the real run
        try:
            import jax
            jax.block_until_ready(jax.numpy.zeros(8))
        except Exception:
            pass
        _CACHE["nc"] = build()
    nc = _CACHE["nc"]
    in_maps = prepare_inputs(x, adj, W_heads, a_heads, W_out, a_out)
    res = run_bass_kernel_spmd(nc, in_maps, list(range(NCORES)))
    out = np.concatenate([res.results[c]["out"] for c in range(NCORES)], axis=0)
    return out.reshape(1, N, NCLS)


# revision 91
# speedup vs baseline: 1.0071x; 1.0071x over previous
"""GAT (2-layer, 3-head) forward on 8 Trainium2 NeuronCores.

Math: with LeakyReLU slope ALPHA=1.0 the edge score e_ij = s1_i + s2_j is
linear, and s1_i cancels inside the row softmax.  The masked softmax over
j therefore reduces to column weights w_j = exp(s2_j - C) restricted to
adj, giving

    h'_i = (sum_j adj_ij * w_j * h_j) / (sum_j adj_ij * w_j)

i.e. one adjacency matmul against G = [w*h | w].  Both GAT layers take
this form (the same adjacency masks both), so the whole network is two
A-matmuls plus small projections.

Sharding: rows of h' (nodes) across 8 cores; each core holds lhsT-layout
adjacency columns A^T[:, slab] and computes its 512-row slab.

Precision (tolerance 2e-2, achieved ~4e-3): the layer-1 adjacency matmul
runs in fp8 e4m3 with DoubleRow perf mode (each matmul contracts a
256-node j-pair at 2x rate); G is scaled by 8, the denominator weights w
by 128 as an fp8 hi/lo pair (hi + lo/16 ~ 8 mantissa bits), with the
scale ratio folded into the reciprocal.  x@W, layer 2 and the epilogues
are bf16.  The edge-score s2, which sits in an exponent, uses a bf16
hi/lo pair of the folded u vector; its softmax max is computed exactly
on the host (negC input), and layer 2 skips max subtraction entirely
(s2' <= ~8, the common exp scale cancels in num/den).

Schedule: batched input DMAs; head-0 x@W + w staging feed gather A
[w-pair cols | G0] so the denominator (riding ct-0's tile loads) and
head-0 column-tiles unblock first, while gather B [G1 | G2] overlaps the
head-0 adjacency matmuls.  h2 accumulates inline per column-tile; g2 is
gathered in two half-slab chunks so the L2 matmul starts on the first
half; the final elu+log_softmax runs batched over all four i-tiles.
"""
import sys

sys.path.insert(0, "/opt/trn_rl_repo")

import numpy as np
import ml_dtypes

import concourse.bass as bass
import concourse.bacc as bacc
import concourse.mybir as mybir
import concourse.bass_isa as bass_isa
import concourse.tile as tile
from concourse.bass_utils import run_bass_kernel_spmd

BF16 = ml_dtypes.bfloat16
F8E4 = ml_dtypes.float8_e4m3

N = 4096
F = 768
HID = 768
NH = 3
NCLS = 256
NCORES = 8
SLAB = N // NCORES          # 512 rows per core
NIT = SLAB // 128           # 4 i-tiles per core
NJT = N // 128              # 32 j-tiles
NFT = F // 128              # 6 f-tiles
NCT = NH * NFT              # 18 feature col-tiles of G
G2C = NCLS + 1              # 257 = classes + s2' column (folded u2)
PAD2 = 264                  # G2 padded to 32B rows
WCOLS = 32                  # w-column slab width (6 used + pad, 32B rows)
GA = WCOLS + HID            # gather-A width: [w cols | head0 G]
GB = 2 * HID                # gather-B width: head1 + head2 G
GH_TOT = NH * HID           # 2304 xcat feature rows of Wo
SG = 8.0                    # fp8 scale on G ( |G*8| << 240 )
SW = 128.0                  # fp8 scale on w (w <= 1)
NJJ = NJT // 2              # 16 j-pair blocks for DoubleRow

AF = mybir.ActivationFunctionType
ALU = mybir.AluOpType


def _enable_ldw_opt():
    # walrus defaults to --enable-ldw-opt=false; with it off every LDWEIGHTS
    # serializes against the previous matmul (~427ns vs ~213ns per 512-col
    # matmul).  Patch the arg builder so the stationary loads pipeline.
    import concourse.bass_utils as _bu
    if getattr(_bu, "_ldw_opt_patched", False):
        return
    _orig = _bu.get_walrus_args

    def _patched(*a, **k):
        args = _orig(*a, **k)
        return [x.replace("--enable-ldw-opt=false", "--enable-ldw-opt=true")
                for x in args]

    _bu.get_walrus_args = _patched
    _bu._ldw_opt_patched = True


def build():
    dt = mybir.dt
    _enable_ldw_opt()
    nc = bacc.Bacc(num_devices=NCORES)

    adjT8_d = nc.dram_tensor("adjT8", [N, SLAB], dt.float8e4, kind="ExternalInput")
    adjT_d = nc.dram_tensor("adjT", [N, SLAB], dt.bfloat16, kind="ExternalInput")
    xTh_d = nc.dram_tensor("xT_hi", [F, SLAB], dt.bfloat16, kind="ExternalInput")
    U6_d = nc.dram_tensor("U6", [F, 8], dt.bfloat16, kind="ExternalInput")
    # negC[0, h] = -max_i s2_i(head h), computed exactly on the host
    negC_d = nc.dram_tensor("negC", [1, NH], dt.float32, kind="ExternalInput")
    W_d = nc.dram_tensor("W", [NH, F, HID], dt.bfloat16, kind="ExternalInput")
    Wo_d = nc.dram_tensor("Wo", [GH_TOT, G2C], dt.bfloat16, kind="ExternalInput")
    out_d = nc.dram_tensor("out", [SLAB, NCLS], dt.float32, kind="ExternalOutput")

    # DRAM scratch + collective buffers (fp8: halves gather + reload bytes)
    gsA = nc.dram_tensor("gsA", [SLAB, GA], dt.float8e4)
    gfA = nc.dram_tensor("gfA", [N, GA], dt.float8e4, addr_space="Shared")
    gsB = nc.dram_tensor("gsB", [SLAB, GB], dt.float8e4)
    gfB = nc.dram_tensor("gfB", [N, GB], dt.float8e4, addr_space="Shared")
    # g2 gathered in two half-slab chunks so the L2 matmul can start on the
    # first half while the second is in flight
    g2_slab = [nc.dram_tensor(f"g2_slab{k}", [SLAB // 2, PAD2], dt.bfloat16)
               for k in range(2)]
    g2_full = [nc.dram_tensor(f"g2_full{k}", [N // 2, PAD2], dt.bfloat16,
                              addr_space="Shared") for k in range(2)]

    rg = [list(range(NCORES))]

    with tile.TileContext(nc) as tc:
      with tc.tile_pool(name="adjt", bufs=NJT) as p_adjt:
        # ---------------- phase 1: s2, w, h=x@W, G build + gathers ----------
        with (
            tc.tile_pool(name="xw", bufs=1) as p_xw,
            tc.tile_pool(name="small", bufs=1) as p_sm,
            tc.tile_pool(name="gtmp", bufs=1) as p_gt,
        ):
            # Batched input loads: one big DMA per tensor (chunked transfers
            # serialize at ~650ns per 128KB, so 70 small DMAs would cost
            # ~45us of serial load time).  x + head-0 W first: they gate
            # s2 and the first x@W matmuls.
            xh_all = p_xw.tile([128, NFT, SLAB], dt.bfloat16, tag="xh", name="xh")
            nc.sync.dma_start(xh_all[:],
                              xTh_d.rearrange("(ft p) i -> p ft i", p=128))

            def xhi(ft, c0, c1):
                return xh_all[:, ft, c0:c1]

            u6 = p_sm.tile([128, NFT, 8], dt.bfloat16, tag="u6", name="u6")
            nc.gpsimd.dma_start(u6[:], U6_d.rearrange("(ft p) c -> p ft c", p=128))
            negC = p_sm.tile([1, NH], dt.float32, tag="negC", name="negC")
            nc.gpsimd.dma_start(negC[:], negC_d[:])
            negCbc = p_sm.tile([128, NH], dt.float32, tag="negCbc", name="negCbc")
            nc.gpsimd.partition_broadcast(negCbc[:], negC[:], channels=128)

            W_t = W_d.rearrange("h (ft p) o -> p h ft o", p=128)
            w0_all = p_xw.tile([128, NFT, HID], dt.bfloat16, tag="w0", name="w0")
            nc.sync.dma_start(w0_all[:], W_t[:, 0])
            w12_all = p_xw.tile([128, 2, NFT, HID], dt.bfloat16, tag="w12",
                                name="w12")
            nc.scalar.dma_start(w12_all[:], W_t[:, 1:3])

            def wsl(h, ft, c0, c1):
                if h == 0:
                    return w0_all[:, ft, c0:c1]
                return w12_all[:, h - 1, ft, c0:c1]

            # fp8 adjacency, j-pair interleaved for DoubleRow (L1 rhs)
            adj8_all = []
            adjT8_t = adjT8_d.rearrange("(half jj i p) n -> half p jj i n",
                                        half=2, i=2, p=128)
            for half in range(2):
                t = p_adjt.tile([128, NJJ // 2, 2, SLAB], dt.float8e4,
                                tag="adj8", name="adj8", bufs=2)
                eng = nc.sync if half == 0 else nc.scalar
                eng.dma_start(t[:], adjT8_t[half])
                adj8_all.append(t)

            def adjd(jj):
                return adj8_all[jj // (NJJ // 2)][:, jj % (NJJ // 2), :, :]

            # bf16 adjacency per original j-tile (L2 lhsT) — needed only at
            # the tail, loaded after the phase-1 traffic
            adjt_all = []
            adjT_t = adjT_d.rearrange("(half jh p) i -> half p jh i",
                                      half=2, p=128)

            def adjs(j, c0=0, c1=SLAB):
                return adjt_all[j // (NJT // 2)][:, j % (NJT // 2), c0:c1]

            # head-0 x@W its 0-2 FIRST (PE warms up, psum tiles park until w
            # is ready), then the tiny s2 matmuls — their DVE chain and the
            # exp overlap the head-0 compute, so gather A fires earlier.
            ctx_psA = tc.tile_pool(name="psA", bufs=3, space="PSUM")
            ps_a = ctx_psA.__enter__()

            def xw_head(h, it):
                ps = ps_a.tile([128, HID], dt.float32, tag="psA", name="psA")
                for ft in range(NFT):
                    xh = xhi(ft, it * 128, (it + 1) * 128)
                    nc.tensor.matmul(ps[:, 0:512], xh, wsl(h, ft, 0, 512),
                                     start=(ft == 0), stop=(ft == NFT - 1))
                    nc.tensor.matmul(ps[:, 512:HID], xh, wsl(h, ft, 512, HID),
                                     start=(ft == 0), stop=(ft == NFT - 1))
                return ps

            h0_ps = [xw_head(0, it) for it in range(NIT - 1)]

            # s2 = x_hi @ (u_hi + u_lo): one PSUM bank, no inter-it reuse
            # stalls.  u kept as a bf16 pair; x_hi-only costs ~0.8% on w,
            # which averages out over ~2k neighbours.
            s2_sb = []
            for h in range(NH):
                s2_sb.append(p_sm.tile([128, NIT], dt.float32, tag="s2",
                                       name="s2", bufs=NH))
            with tc.tile_pool(name="psS", bufs=1, space="PSUM") as ps_s:
                p6 = ps_s.tile([128, NIT, 8], dt.float32, tag="p6", name="p6")
                for it in range(NIT):
                    for ft in range(NFT):
                        xh = xhi(ft, it * 128, (it + 1) * 128)
                        nc.tensor.matmul(p6[:, it, :], xh, u6[:, ft, :],
                                         start=(ft == 0), stop=(ft == NFT - 1))
                for it in range(NIT):
                    t6 = p_sm.tile([128, 8], dt.float32, tag="t6", name="t6",
                                   bufs=2)
                    nc.vector.tensor_copy(t6[:], p6[:, it, :])
                    tsum = p_sm.tile([128, NH], dt.float32, tag="tsum",
                                     name="tsum", bufs=2)
                    nc.vector.tensor_tensor(tsum[:], t6[:, 0:2 * NH:2],
                                            t6[:, 1:2 * NH:2], ALU.add)
                    for h in range(NH):
                        nc.vector.tensor_copy(s2_sb[h][:, it:it + 1],
                                              tsum[:, h:h + 1])

            # w = exp(s2 - C) with the host-computed C — no collective needed.
            # Stage w*SW as an fp8 hi/lo pair (hi + lo/16 ≈ 8 mantissa bits)
            # for the DoubleRow denominator matmul, and keep w*SG in fp32 for
            # scaling G.
            w_sb, w8_sb = [], []
            for h in range(NH):
                w = p_sm.tile([128, NIT], dt.float32, tag="wexp", name="wexp",
                              bufs=NH)
                nc.scalar.activation(w[:], s2_sb[h][:], AF.Exp,
                                     bias=negCbc[:, h:h + 1])
                w_sb.append(w)
                w8 = p_sm.tile([128, NIT], dt.float32, tag="wsg", name="wsg",
                               bufs=NH)
                nc.vector.tensor_scalar_mul(w8[:], w[:], SG)
                w8_sb.append(w8)
            whi3 = p_sm.tile([128, NH, NIT], dt.float8e4, tag="whi3",
                             name="whi3")
            wlo3 = p_sm.tile([128, NH, NIT], dt.float8e4, tag="wlo3",
                             name="wlo3")
            for h in range(NH):
                wsw = p_sm.tile([128, NIT], dt.float32, tag="wsw", name="wsw",
                                bufs=2)
                nc.vector.tensor_scalar_mul(wsw[:], w_sb[h][:], SW)
                nc.vector.tensor_copy(whi3[:, h, :], wsw[:])
                wr = p_sm.tile([128, NIT], dt.float32, tag="wr", name="wr",
                               bufs=2)
                nc.vector.tensor_tensor(wr[:], wsw[:], whi3[:, h, :],
                                        ALU.subtract)
                nc.vector.tensor_scalar_mul(wlo3[:, h, :], wr[:], 16.0)
            for it in range(NIT):
                rows = slice(it * 128, (it + 1) * 128)
                wt = p_sm.tile([128, WCOLS], dt.float8e4, tag="wt", name="wt",
                               bufs=2)
                nc.vector.memset(wt[:], 0.0)
                nc.vector.tensor_copy(wt[:, 0:NH], whi3[:, :, it])
                nc.vector.tensor_copy(wt[:, NH:2 * NH], wlo3[:, :, it])
                nc.gpsimd.dma_start(gsA[rows, 0:WCOLS], wt[:])

            # head-0 G build -> gather A fires as early as possible
            for it in range(NIT - 1):
                g = p_gt.tile([128, HID], dt.float8e4, tag="g0",
                              name="g0", bufs=4)
                nc.vector.tensor_scalar_mul(g[:], h0_ps[it][:],
                                            w8_sb[0][:, it:it + 1])
                rows = slice(it * 128, (it + 1) * 128)
                nc.sync.dma_start(gsA[rows, WCOLS:GA], g[:])
            ps = xw_head(0, NIT - 1)
            g = p_gt.tile([128, HID], dt.float8e4, tag="g0", name="g0", bufs=4)
            nc.vector.tensor_scalar_mul(g[:], ps[:], w8_sb[0][:, NIT - 1:NIT])
            nc.sync.dma_start(gsA[(NIT - 1) * 128:SLAB, WCOLS:GA], g[:])
            nc.gpsimd.collective_compute(
                "AllGather", ALU.bypass, replica_groups=rg,
                ins=[gsA[:]], outs=[gfA[:]])
            h0_ps = None

            # heads 1-2: x@W, scale, stage, gather B
            for h in (1, 2):
                for it in range(NIT):
                    ps = xw_head(h, it)
                    g = p_gt.tile([128, HID], dt.float8e4, tag="g0",
                                  name="g0", bufs=4)
                    nc.vector.tensor_scalar_mul(g[:], ps[:],
                                                w8_sb[h][:, it:it + 1])
                    rows = slice(it * 128, (it + 1) * 128)
                    eng = nc.sync if h == 1 else nc.scalar
                    eng.dma_start(gsB[rows, (h - 1) * HID:h * HID], g[:])
            nc.gpsimd.collective_compute(
                "AllGather", ALU.bypass, replica_groups=rg,
                ins=[gsB[:]], outs=[gfB[:]])
            ctx_psA.__exit__(None, None, None)
            # bf16 adjacency for the L2 lhsT — queue after the G staging
            for half in range(2):
                t = p_adjt.tile([128, NJT // 2, SLAB], dt.bfloat16, tag="adjt",
                                name="adjt", bufs=2)
                eng = nc.sync if half == 0 else nc.scalar
                eng.dma_start(t[:], adjT_t[half])
                adjt_all.append(t)

        # ---------------- L1 adjacency matmul + epilogue + layer 2 ----------
        with tc.tile_pool(name="xct", bufs=1) as p_xct:
            with (
                tc.tile_pool(name="gst", bufs=8) as p_gst,
                tc.tile_pool(name="etmp", bufs=1) as p_et,
                tc.tile_pool(name="wo", bufs=1) as p_wo,
                tc.tile_pool(name="l2a", bufs=1) as p_l2a,
                tc.tile_pool(name="ps1", bufs=4, space="PSUM") as ps_1,
                tc.tile_pool(name="psh2", bufs=4, space="PSUM") as ps_h2,
            ):
                # Wo loads early; they only feed the inline h2 matmuls
                wo_sb = []
                Wo_t = Wo_d.rearrange("(ot p) c -> ot p c", p=128)
                for ot in range(NCT):
                    t = p_wo.tile([128, G2C], dt.bfloat16, tag="wo", name="wo",
                                  bufs=NCT)
                    eng = nc.sync if ot % 2 == 0 else nc.scalar
                    eng.dma_start(t[:], Wo_t[ot])
                    wo_sb.append(t)

                # feature col-tiles, head-major; epilogue + h2 inline per ct.
                # DoubleRow fp8: each matmul contracts a j-PAIR (256 nodes).
                # ct 0 also carries the w columns (first WCOLS of gfA), so the
                # denominator matmuls ride its tile loads — no separate
                # strided gather of w.
                gvA = gfA.rearrange("(jb jj i p) c -> jb p jj i c",
                                    jj=2, i=2, p=128)
                gvB = gfB.rearrange("(jb jj i p) c -> jb p jj i c",
                                    jj=2, i=2, p=128)
                ps2l = [ps_h2.tile([128, G2C], dt.float32, tag="psh2",
                                   name="psh2") for _ in range(NIT)]
                rbc = [None] * NH
                psd = ps_1.tile([NH, 2, SLAB], dt.float32, tag="psd",
                                name="psd", bufs=1)
                DR = mybir.MatmulPerfMode.DoubleRow
                for cp in range(NCT // 2):
                    h = (2 * cp) // NFT
                    lp = cp % (NFT // 2)
                    pss = [ps_1.tile([128, SLAB], dt.float32, tag="ps1",
                                     name="ps1", bufs=2) for _ in range(2)]
                    for jb in range(NJJ // 2):
                        # one load covers BOTH cts of the pair (256B chunks)
                        if cp == 0:
                            gt = p_gst.tile([128, 2, 2, WCOLS + 256],
                                            dt.float8e4, tag="gst0",
                                            name="gst0", bufs=8)
                            eng = nc.sync if jb % 2 == 0 else nc.scalar
                            eng.dma_start(gt[:], gvA[jb, :, :, :, 0:WCOLS + 256])
                            goff = WCOLS
                        elif h == 0:
                            gt = p_gst.tile([128, 2, 2, 256], dt.float8e4,
                                            tag="gst", name="gst")
                            eng = nc.sync if jb % 2 == 0 else nc.scalar
                            eng.dma_start(gt[:], gvA[jb, :, :, :,
                                                     WCOLS + lp * 256:
                                                     WCOLS + (lp + 1) * 256])
                            goff = 0
                        else:
                            gt = p_gst.tile([128, 2, 2, 256], dt.float8e4,
                                            tag="gst", name="gst")
                            eng = nc.sync if jb % 2 == 0 else nc.scalar
                            c0 = (h - 1) * HID + lp * 256
                            eng.dma_start(gt[:], gvB[jb, :, :, :, c0:c0 + 256])
                            goff = 0
                        for q in range(2):
                            jj = jb * 2 + q
                            if cp == 0:
                                nc.tensor.matmul(psd[:, 0, :],
                                                 gt[:, q, :, 0:NH],
                                                 adjd(jj), start=(jj == 0),
                                                 stop=(jj == NJJ - 1),
                                                 perf_mode=DR)
                                nc.tensor.matmul(psd[:, 1, :],
                                                 gt[:, q, :, NH:2 * NH],
                                                 adjd(jj), start=(jj == 0),
                                                 stop=(jj == NJJ - 1),
                                                 perf_mode=DR)
                            for s in range(2):
                                o = goff + s * 128
                                nc.tensor.matmul(pss[s][:],
                                                 gt[:, q, :, o:o + 128],
                                                 adjd(jj), start=(jj == 0),
                                                 stop=(jj == NJJ - 1),
                                                 perf_mode=DR)
                    if cp == 0:
                        # den = (psd_hi + psd_lo/16); recip carries SW/SG
                        dlo = p_et.tile([NH, SLAB], dt.float32, tag="dlo",
                                        name="dlo")
                        nc.vector.tensor_copy(dlo[:], psd[:, 1, :])
                        den3 = p_et.tile([NH, SLAB], dt.float32, tag="den3",
                                         name="den3")
                        nc.vector.scalar_tensor_tensor(den3[:], dlo[:],
                                                       1.0 / 16.0, psd[:, 0, :],
                                                       ALU.mult, ALU.add)
                        recip3 = p_et.tile([NH, SLAB], dt.float32, tag="recip3",
                                           name="recip3")
                        nc.vector.reciprocal(recip3[:], den3[:])
                        nc.vector.tensor_scalar_mul(recip3[:], recip3[:],
                                                    SW / SG)
                        for hh in range(NH):
                            rrow = p_et.tile([1, SLAB], dt.float32, tag="rrow",
                                             name="rrow", bufs=2)
                            nc.sync.dma_start(rrow[:], recip3[hh:hh + 1, :])
                            rb = p_et.tile([128, SLAB], dt.float32, tag="rbc",
                                           name="rbc", bufs=NH)
                            nc.gpsimd.partition_broadcast(rb[:], rrow[:],
                                                          channels=128)
                            rbc[hh] = rb
                    for s in range(2):
                        ct = 2 * cp + s
                        # xcatT tile = elu(numT / den), bf16
                        z = p_et.tile([128, SLAB], dt.float32, tag="z",
                                      name="z", bufs=2)
                        nc.vector.tensor_tensor(z[:], pss[s][:], rbc[h][:],
                                                ALU.mult)
                        e = p_et.tile([128, SLAB], dt.float32, tag="e",
                                      name="e", bufs=2)
                        nc.scalar.activation(e[:], z[:], AF.Exp)
                        nc.vector.tensor_scalar(e[:], e[:], 1.0, -1.0, ALU.min,
                                                ALU.add)
                        xc = p_xct.tile([128, SLAB], dt.bfloat16, tag="xcp",
                                        name="xcp", bufs=NCT)
                        nc.vector.scalar_tensor_tensor(xc[:], z[:], 0.0, e[:],
                                                       ALU.max, ALU.add)
                        # layer 2 accumulation: h2 += xcat_ct @ Wo_ct
                        for it in range(NIT):
                            nc.tensor.matmul(ps2l[it][:],
                                             xc[:, it * 128:(it + 1) * 128],
                                             wo_sb[ct][:],
                                             start=(ct == 0),
                                             stop=(ct == NCT - 1))

                # layer-2 weights w2 = exp(s2') with NO max subtraction:
                # s2' stays well under fp32/bf16 exp range and the common
                # scale cancels exactly in num/den.
                for it in range(NIT):
                    rows = slice((it % 2) * 128, (it % 2 + 1) * 128)
                    w2 = p_l2a.tile([128, 1], dt.float32, tag="w2", name="w2",
                                    bufs=2)
                    nc.scalar.activation(w2[:], ps2l[it][:, NCLS:G2C], AF.Exp)
                    g2b = p_l2a.tile([128, PAD2], dt.bfloat16, tag="g2b",
                                     name="g2b", bufs=2)
                    nc.vector.tensor_scalar_mul(g2b[:, 0:NCLS],
                                                ps2l[it][:, 0:NCLS], w2[:])
                    nc.vector.tensor_copy(g2b[:, NCLS:G2C], w2[:])
                    nc.vector.memset(g2b[:, G2C:PAD2], 0.0)
                    nc.sync.dma_start(g2_slab[it // 2][rows, :], g2b[:])
                    if it == 1:
                        nc.gpsimd.collective_compute(
                            "AllGather", ALU.bypass, replica_groups=rg,
                            ins=[g2_slab[0][:]], outs=[g2_full[0][:]])
                nc.gpsimd.collective_compute(
                    "AllGather", ALU.bypass, replica_groups=rg,
                    ins=[g2_slab[1][:]], outs=[g2_full[1][:]])

            # L2 adjacency matmul + final epilogue
            with (
                tc.tile_pool(name="g2t", bufs=NJT) as p_g2t,
                tc.tile_pool(name="fin", bufs=1) as p_f,
                tc.tile_pool(name="ps2", bufs=4, space="PSUM") as ps_2,
            ):
                g2tiles = []
                for k in range(2):
                    g2v = g2_full[k].rearrange("(tb t p) c -> tb p t c",
                                               tb=2, p=128)
                    for tb in range(2):
                        gt2 = p_g2t.tile([128, 8, PAD2], dt.bfloat16,
                                         tag="g2t", name="g2t", bufs=4)
                        eng = nc.sync if tb % 2 == 0 else nc.scalar
                        eng.dma_start(gt2[:], g2v[tb])
                        g2tiles.append(gt2)
                # one psum tile, 512-col (bank-aligned) stride per it
                ps2 = ps_2.tile([128, NIT, 512], dt.float32, tag="ps2",
                                name="ps2", bufs=1)
                # k=0 half for all i-tiles while gather B is in flight
                for it in range(NIT):
                    for t in range(NJT // 2):
                        jt = (t // 2) * 4 + (t % 2)
                        lhs = adjs(jt, it * 128, (it + 1) * 128)
                        nc.tensor.matmul(ps2[:, it, 0:G2C], lhs,
                                         g2tiles[t // 8][:, t % 8, 0:G2C],
                                         start=(t == 0), stop=False)
                # k=1 half it-by-it, epilogue inlined per i-tile so the
                # elu+log_softmax chain overlaps the remaining matmuls
                for it in range(NIT):
                    for t in range(NJT // 2):
                        jt = (t // 2) * 4 + 2 + (t % 2)
                        lhs = adjs(jt, it * 128, (it + 1) * 128)
                        nc.tensor.matmul(ps2[:, it, 0:G2C], lhs,
                                         g2tiles[2 + t // 8][:, t % 8, 0:G2C],
                                         start=False, stop=(t == NJT // 2 - 1))
                    r2 = p_f.tile([128, 1], dt.float32, tag="r2", name="r2",
                                  bufs=2)
                    nc.vector.reciprocal(r2[:], ps2[:, it, NCLS:G2C])
                    z = p_f.tile([128, NCLS], dt.float32, tag="z2", name="z2",
                                 bufs=2)
                    nc.vector.tensor_scalar_mul(z[:], ps2[:, it, 0:NCLS],
                                                r2[:])
                    e = p_f.tile([128, NCLS], dt.float32, tag="e2", name="e2",
                                 bufs=2)
                    nc.scalar.activation(e[:], z[:], AF.Exp)
                    nc.vector.tensor_scalar(e[:], e[:], 1.0, -1.0, ALU.min,
                                            ALU.add)
                    o = p_f.tile([128, NCLS], dt.float32, tag="o2", name="o2",
                                 bufs=2)
                    nc.vector.scalar_tensor_tensor(o[:], z[:], 0.0, e[:],
                                                   ALU.max, ALU.add)
                    # log_softmax without max subtraction (o <= ~10)
                    t4 = p_f.tile([128, NCLS], dt.float32, tag="t4", name="t4",
                                  bufs=2)
                    nc.scalar.activation(t4[:], o[:], AF.Exp)
                    ssum = p_f.tile([128, 1], dt.float32, tag="ssum",
                                    name="ssum", bufs=2)
                    nc.vector.tensor_reduce(ssum[:], t4[:],
                                            axis=mybir.AxisListType.X,
                                            op=ALU.add)
                    lg = p_f.tile([128, 1], dt.float32, tag="lg", name="lg",
                                  bufs=2)
                    nc.scalar.activation(lg[:], ssum[:], AF.Ln)
                    fin = p_f.tile([128, NCLS], dt.float32, tag="fin",
                                   name="fin", bufs=2)
                    nc.vector.tensor_scalar(fin[:], o[:], lg[:], None,
                                            ALU.subtract)
                    nc.sync.dma_start(out_d[it * 128:(it + 1) * 128, :],
                                      fin[:])

    nc.finalize()
    return nc


_CACHE = {}


def _pair(a):
    hi = a.astype(BF16)
    lo = (a - hi.astype(np.float32)).astype(BF16)
    return hi, lo


def prepare_inputs(x, adj, W_heads, a_heads, W_out, a_out):
    """Shard + lay out the full inputs for the 8 cores."""
    x2 = np.asarray(x, np.float32)[0]          # [N, F]
    adj2 = np.asarray(adj)[0]                  # [N, N] int32
    W3 = np.asarray(W_heads, np.float32).reshape(NH, F, HID)
    a3 = np.asarray(a_heads, np.float32)       # [NH, 2*HID, 1]
    Wo = np.asarray(W_out, np.float32).reshape(GH_TOT, NCLS)
    ao = np.asarray(a_out, np.float32)         # [2*NCLS, 1]

    # fold the edge-score projections into the weights:
    #   s2 = x @ (W @ a2),   s2' = xcat @ (Wo @ ao2)
    u = np.einsum("hfo,ho->hf", W3.astype(np.float64),
                  a3[:, HID:, 0].astype(np.float64)).astype(np.float32)  # [NH,F]
    u_hi, u_lo = _pair(u)
    U6 = np.zeros((F, 8), BF16)
    for h in range(NH):
        U6[:, 2 * h] = u_hi[h]
        U6[:, 2 * h + 1] = u_lo[h]
    u2 = (Wo.astype(np.float64) @ ao[NCLS:, 0].astype(np.float64)).astype(np.float32)
    Wo_ext = np.concatenate([Wo, u2[:, None]], axis=1)       # [GH, 257]
    Wo_b = Wo_ext.astype(BF16)
    W_b = W3.astype(BF16)
    xT = np.ascontiguousarray(x2.T)            # [F, N]
    adjb = adj2.astype(BF16)                   # exact 0/1

    # exact per-head max of s2 = x @ u, folded on the host so the device
    # needs no max-reduction collective.  Mirror the device arithmetic
    # (bf16 x_hi against the u hi/lo pair, accumulated in fp32).
    xh_f = x2.astype(BF16).astype(np.float32)
    s2 = (xh_f @ u_hi.T.astype(np.float32)
          + xh_f @ u_lo.T.astype(np.float32))                     # [N, NH]
    negC = -s2.max(axis=0, keepdims=True).astype(np.float32)      # [1, NH]

    in_maps = []
    for c in range(NCORES):
        sl = slice(c * SLAB, (c + 1) * SLAB)
        xh = np.ascontiguousarray(xT[:, sl]).astype(BF16)
        adjTc = np.ascontiguousarray(adjb[sl, :].T)
        in_maps.append({
            "adjT": adjTc,
            "adjT8": adjTc.astype(F8E4),
            "xT_hi": xh,
            "U6": U6, "negC": negC,
            "W": W_b, "Wo": Wo_b,
        })
    return in_maps


def kernel(x, adj, W_heads, a_heads, W_out, a_out):
    if "nc" not in _CACHE:
        # touch the devices once so any residual bad state from a previous
        # process surfaces (and clears) before the real run
        try:
            import jax
            jax.block_until_ready(jax.numpy.zeros(8))
        except Exception:
            pass
        _CACHE["nc"] = build()
    nc = _CACHE["nc"]
    in_maps = prepare_inputs(x, adj, W_heads, a_heads, W_out, a_out)
    res = run_bass_kernel_spmd(nc, in_maps, list(range(NCORES)))
    out = np.concatenate([res.results[c]["out"] for c in range(NCORES)], axis=0)
    return out.reshape(1, N, NCLS)
